# revision 21
# baseline (speedup 1.0000x reference)
"""Trainium2 Bass kernel for MultiHeadLinearAttention (RALA / tanh-kernel linear attention).

Reference computation (B=4, L=8192, E=512, H=8, D=64):
    phi_Q = tanh(Q @ Wq.T + bq) + 1 ;  phi_K = tanh(K @ Wk.T + bk) + 1
    q_global = mean_L(Qh) / sqrt(D)
    alpha = softmax_L(q_global . Kh) * L
    KV[h]  = sum_l (phi_Kh*alpha) (x) Vh ;  K_sum[h] = sum_l phi_Kh*alpha
    out = (phi_Qh @ KV) / (phi_Qh . K_sum + eps)

Sharding: 8 cores = (batch b in 0..3) x (sequence half in 0..1).  Every
input element is read exactly once.  The L-reductions (q_global, KV,
K_sum) are completed with tiny pairwise AllReduces between the two cores
sharing a batch.

Identities used on-chip:
    tanh(x+b)+1 = 2*sigmoid(2x+2b)      (single ACT pass from PSUM)
    exp(x) = s/(1-s) with s=sigmoid(x)  (stays on the sigmoid table set)
    The per-head scalars (the 2x factors and the softmax normalizer
    L/S_h) multiply BOTH num and den, so they cancel in out=num/den and
    are never computed.  eps=1e-6 is dropped: den ~ 1e5 before any
    normalization, eps is far below its fp32 ulp.
    softmax max-subtraction is skipped: |scores| ~ 0.01.

Compute dtype: bf16 matmul inputs with fp32 PSUM accumulation.
"""

import numpy as np
import ml_dtypes

import concourse.bass as bass
from concourse import bacc
import concourse.mybir as mybir
import concourse.tile as tile
from concourse.bass_utils import run_bass_kernel_spmd
from concourse.masks import make_identity

B, L, E, H, D = 4, 8192, 512, 8, 64
N_CORES = 8
LH = L // 2            # 4096 rows per core
NLB = LH // 128        # 32 l-blocks of 128 rows
NCH = LH // 512        # 8 chunks of 512 rows
NEB = E // 128         # 4 e-blocks (= 4 head-pairs = 4 j-blocks)
GROUPS = [[0, 1], [2, 3], [4, 5], [6, 7]]
F32 = mybir.dt.float32
BF16 = mybir.dt.bfloat16
AF = mybir.ActivationFunctionType
ALU = mybir.AluOpType
QG_SCALE = 1.0 / (L * np.sqrt(D))   # scores = (sum_l Q) . K * QG_SCALE

DEBUG = False


def _build():
    nc = bacc.Bacc("TRN2", target_bir_lowering=False, debug=False,
                   num_devices=N_CORES)
    Qs = nc.dram_tensor("Qs", [LH, E], F32, kind="ExternalInput")
    Ks = nc.dram_tensor("Ks", [LH, E], F32, kind="ExternalInput")
    Vs = nc.dram_tensor("Vs", [LH, E], F32, kind="ExternalInput")
    WqT = nc.dram_tensor("WqT", [E, E], BF16, kind="ExternalInput")
    WkT = nc.dram_tensor("WkT", [E, E], BF16, kind="ExternalInput")
    bq2 = nc.dram_tensor("bq2", [E], F32, kind="ExternalInput")
    bk2 = nc.dram_tensor("bk2", [1, E], BF16, kind="ExternalInput")
    Out = nc.dram_tensor("Out", [LH, E], F32, kind="ExternalOutput")

    # DRAM bounce buffers for the two pairwise AllReduces
    qg_in = nc.dram_tensor("qg_in", [1, E], F32)
    qg_out = nc.dram_tensor("qg_out", [1, E], F32)
    kv_in = nc.dram_tensor("kv_in", [129, E], F32)
    kv_out = nc.dram_tensor("kv_out", [129, E], F32)
    dbg = None
    if DEBUG:
        dbg = {
            "dbg_qg": nc.dram_tensor("dbg_qg", [1, E], F32, kind="ExternalOutput"),
            "dbg_sigqt0": nc.dram_tensor("dbg_sigqt0", [128, LH], BF16, kind="ExternalOutput"),
            "dbg_sigk0": nc.dram_tensor("dbg_sigk0", [128, E], BF16, kind="ExternalOutput"),
            "dbg_expt": nc.dram_tensor("dbg_expt", [128, NLB * H], F32, kind="ExternalOutput"),
            "dbg_kvout": nc.dram_tensor("dbg_kvout", [129, E], F32, kind="ExternalOutput"),
        }

    import contextlib
    with tile.TileContext(nc) as tc:
        with contextlib.ExitStack() as ctx:
            _kernel_body(nc, tc, ctx, Qs, Ks, Vs, WqT, WkT, bq2, bk2, Out,
                         qg_in, qg_out, kv_in, kv_out, dbg)
    nc.finalize()
    return nc


def _kernel_body(nc, tc, ctx, Qs, Ks, Vs, WqT, WkT, bq2, bk2, Out,
                 qg_in, qg_out, kv_in, kv_out, dbg=None):
    enter = ctx.enter_context

    # ---------------- pools ----------------
    singles = enter(tc.tile_pool(name="singles", bufs=1))
    stream = enter(tc.tile_pool(name="stream", bufs=6))       # Q/K/V nat tiles
    qt_pool = enter(tc.tile_pool(name="qt", bufs=2))          # QT chunk tiles
    outp = enter(tc.tile_pool(name="outp", bufs=6))           # out tiles
    recp = enter(tc.tile_pool(name="recp", bufs=8))
    misc = enter(tc.tile_pool(name="misc", bufs=1))
    # PSUM budget (8 banks):  big 2 + tp 2 + small 2 + kv 1 + accum1 1
    ps_big = enter(tc.tile_pool(name="ps_big", bufs=2, space="PSUM"))
    ps_tp = enter(tc.tile_pool(name="ps_tp", bufs=2, space="PSUM"))
    ps_small = enter(tc.tile_pool(name="ps_small", bufs=2, space="PSUM"))
    ps_kv = enter(tc.tile_pool(name="ps_kv", bufs=1, space="PSUM"))
    ps_acc = enter(tc.tile_pool(name="ps_acc", bufs=1, space="PSUM"))

    # ---------------- constants ----------------
    ident = singles.tile([128, 128], BF16, tag="ident")
    make_identity(nc, ident)
    ones_col = singles.tile([128, 1], BF16, tag="ones_col")
    nc.vector.memset(ones_col, 1.0)
    ones_row = singles.tile([1, 128], BF16, tag="ones_row")
    nc.vector.memset(ones_row, 1.0)

    # weights / biases
    wq_sb = []
    wk_sb = []
    for eb in range(NEB):
        wq = singles.tile([128, E], BF16, tag=f"wq{eb}", name=f"wq{eb}")
        nc.sync.dma_start(out=wq[:, :], in_=WqT.ap()[eb * 128:(eb + 1) * 128, :])
        wq_sb.append(wq)
        wk = singles.tile([128, E], BF16, tag=f"wk{eb}", name=f"wk{eb}")
        nc.sync.dma_start(out=wk[:, :], in_=WkT.ap()[eb * 128:(eb + 1) * 128, :])
        wk_sb.append(wk)
    bq2_sb = singles.tile([128, NEB], F32, tag="bq2")
    nc.sync.dma_start(
        out=bq2_sb[:, :],
        in_=bass.AP(tensor=bq2, offset=0, ap=[[1, 128], [128, NEB]]))
    bk2_sb = singles.tile([1, E], BF16, tag="bk2")
    nc.sync.dma_start(out=bk2_sb[:, :], in_=bk2.ap()[:, :])

    # ---------------- persistent SBUF tensors ----------------
    # sigQT[jb]: [128 j, LH l] bf16 ; KT[eb]: [128 e, LH l] bf16
    sigQT = [singles.tile([128, LH], BF16, tag=f"sigqt{j}", name=f"sigqt{j}")
             for j in range(NEB)]
    KT = [singles.tile([128, LH], BF16, tag=f"kt{e}", name=f"kt{e}")
          for e in range(NEB)]
    sigK = singles.tile([128, NLB * E], BF16, tag="sigk")   # [l, 512 j] per blk
    expS = singles.tile([128, NLB * H], F32, tag="exps")    # sigmoid(scores)
    expT_b = singles.tile([128, NLB * H], BF16, tag="exptb")
    # qg partials: accum_out of the QT-copy per (eb, chunk)
    qg_acc = [singles.tile([128, NCH], F32, tag=f"qgacc{e}", name=f"qgacc{e}")
              for e in range(NEB)]

    # ---------------- phase 1: Q pass ----------------
    # K loads are emitted interleaved with the Q chunks (kn has 32 bufs,
    # so all of K prefetches during phase 1 and the K compute can start
    # the moment the last phi_Q matmul drains).
    kns_all = []
    for ci in range(NCH):
        qns = []
        for lb in range(4):
            lbg = ci * 4 + lb
            qn = stream.tile([128, E], BF16, tag="qn")
            nc.gpsimd.dma_start(out=qn[:, :],
                                in_=Qs.ap()[lbg * 128:(lbg + 1) * 128, :])
            qns.append(qn)
        kns = []
        for lb in range(4):
            lbg = ci * 4 + lb
            kn = stream.tile([128, E], BF16, tag="kn", bufs=32)
            nc.gpsimd.dma_start(out=kn[:, :],
                                in_=Ks.ap()[lbg * 128:(lbg + 1) * 128, :])
            kns.append(kn)
        kns_all.append(kns)
        qt_tiles = []
        for eb in range(NEB):
            tp = ps_tp.tile([128, 512], BF16, tag="tp")
            for lb in range(4):
                nc.tensor.transpose(tp[:, lb * 128:(lb + 1) * 128],
                                    qns[lb][:, eb * 128:(eb + 1) * 128],
                                    ident[:, :])
            qt = qt_pool.tile([128, 512], BF16, tag=f"qt{eb}", name=f"qtt{eb}")
            # copy PSUM->SBUF on DVE; accum_out gives sum over l for free,
            # accumulating the q_global partial per (e, chunk).
            nc.vector.tensor_scalar(
                out=qt[:, :], in0=tp[:, :], scalar1=0.0, scalar2=0.0,
                op0=ALU.add, op1=ALU.add,
                accum_out=qg_acc[eb][:, ci:ci + 1])
            qt_tiles.append(qt)
        for jb in range(NEB):
            pp = ps_big.tile([128, 512], F32, tag="big")
            for eb in range(NEB):
                nc.tensor.matmul(pp[:, :],
                                 wq_sb[eb][:, jb * 128:(jb + 1) * 128],
                                 qt_tiles[eb][:, :],
                                 start=(eb == 0), stop=(eb == NEB - 1))
            nc.scalar.activation(
                out=sigQT[jb][:, ci * 512:(ci + 1) * 512], in_=pp[:, :],
                func=AF.Sigmoid, bias=bq2_sb[:, jb:jb + 1], scale=2.0)

    # ---------------- phase 2: qg exchange ----------------
    for eb in range(NEB):
        qgc = misc.tile([128, 1], F32, tag=f"qgc{eb}", name=f"qgc{eb}")
        nc.vector.reduce_sum(qgc[:, :], qg_acc[eb][:, :],
                             axis=mybir.AxisListType.X)
        nc.sync.dma_start(
            out=qg_in.ap()[0:1, eb * 128:(eb + 1) * 128].rearrange(
                "a b -> b a"),
            in_=qgc[:, :])
    nc.gpsimd.collective_compute(
        "AllReduce", ALU.add, replica_groups=GROUPS,
        ins=[qg_in.ap().opt()], outs=[qg_out.ap().opt()])
    qgbd = [None] * NEB

    def build_qgbd():
        # blockdiag q_global stationaries [128, 2] bf16 per head-pair.
        # Loads go over HWDGE (sync) so they don't head-of-line-block the
        # gpsimd queue carrying the V cast-loads; DVE does the f32->bf16.
        qgbd_f = misc.tile([128, 2 * NEB], F32, tag="qgbdf", name="qgbdf")
        for p in range(NEB):
            nc.sync.dma_start(
                out=qgbd_f[0:64, 2 * p:2 * p + 1],
                in_=qg_out.ap()[0:1, p * 128:p * 128 + 64].rearrange(
                    "a b -> b a"))
            nc.sync.dma_start(
                out=qgbd_f[64:128, 2 * p + 1:2 * p + 2],
                in_=qg_out.ap()[0:1, p * 128 + 64:(p + 1) * 128].rearrange(
                    "a b -> b a"))
            t = singles.tile([128, 2], BF16, tag=f"qgbd{p}", name=f"qgbd{p}")
            nc.vector.memset(t, 0.0)
            nc.vector.tensor_copy(t[0:64, 0:1], qgbd_f[0:64, 2 * p:2 * p + 1])
            nc.vector.tensor_copy(t[64:128, 1:2],
                                  qgbd_f[64:128, 2 * p + 1:2 * p + 2])
            qgbd[p] = t

    # ------- phases 3-6 fused: K/V streaming pass -------
    # Main stage per chunk: transposes -> KT, phi_K.  The qg-dependent
    # stage (scoresT, exp, phiKs scale, V/KV accumulation) is emitted
    # DEFER chunks later so the engine queues never stall on the qg
    # AllReduce.
    DEFER = 3
    kv_psum = ps_kv.tile([128, E], F32, tag="kv")
    ksum_psum = ps_acc.tile([1, E], F32, tag="acc1")

    def deferred_stage(ci):
        for lb in range(4):
            lbg = ci * 4 + lb
            sc = ps_small.tile([128, H], F32, tag="small", name="sc")
            for p in range(NEB):
                nc.tensor.matmul(sc[:, 2 * p:2 * p + 2],
                                 KT[p][:, lbg * 128:(lbg + 1) * 128],
                                 qgbd[p][:, :], start=True, stop=True,
                                 skip_group_check=True)
            nc.scalar.activation(out=expS[:, lbg * H:(lbg + 1) * H],
                                 in_=sc[:, :], func=AF.Sigmoid,
                                 scale=float(QG_SCALE))
        # exp = s/(1-s) for the whole chunk [128, 32]
        cs = slice(ci * 4 * H, (ci + 1) * 4 * H)
        om = misc.tile([128, 4 * H], F32, tag="om", name="om")
        nc.vector.tensor_scalar(out=om[:, :], in0=expS[:, cs], scalar1=-1.0,
                                scalar2=1.0, op0=ALU.mult, op1=ALU.add)
        nc.vector.reciprocal(om[:, :], om[:, :])
        nc.vector.tensor_mul(expT_b[:, cs], expS[:, cs], om[:, :])
        # phiKs = sigK * expT for the whole chunk (one strided TT)
        sig_sl = sigK[:, ci * 4 * E:(ci + 1) * 4 * E].rearrange(
            "p (t h d) -> p t h d", t=4, h=H)
        eb_ap = expT_b[:, cs].rearrange("p (t h) -> p t h", t=4).unsqueeze(
            3).broadcast_to([128, 4, H, D])
        nc.vector.tensor_mul(sig_sl, sig_sl, eb_ap)
        # V / KV accumulation for this chunk
        for lb in range(4):
            lbg = ci * 4 + lb
            vn = stream.tile([128, E], BF16, tag="vn", name="vn")
            nc.gpsimd.dma_start(out=vn[:, :],
                                in_=Vs.ap()[lbg * 128:(lbg + 1) * 128, :])
            for p in range(NEB):
                # start=True clears has_written for the WHOLE bank: only the
                # very first matmul into this bank may use it.
                nc.tensor.matmul(
                    kv_psum[:, p * 128:(p + 1) * 128],
                    sigK[:, lbg * E + p * 128:lbg * E + (p + 1) * 128],
                    vn[:, p * 128:(p + 1) * 128],
                    start=(lbg == 0 and p == 0), stop=(lbg == NLB - 1),
                    skip_group_check=True)
            nc.tensor.matmul(ksum_psum[:, :], ones_col[:, :],
                             sigK[:, lbg * E:(lbg + 1) * E],
                             start=(lbg == 0), stop=(lbg == NLB - 1),
                             skip_group_check=True)

    for ci in range(NCH):
        kns = kns_all[ci]
        for eb in range(NEB):
            tp = ps_tp.tile([128, 512], BF16, tag="tp")
            for lb in range(4):
                nc.tensor.transpose(tp[:, lb * 128:(lb + 1) * 128],
                                    kns[lb][:, eb * 128:(eb + 1) * 128],
                                    ident[:, :])
            nc.vector.tensor_copy(KT[eb][:, ci * 512:(ci + 1) * 512], tp[:, :])
        for lb in range(4):
            lbg = ci * 4 + lb
            pp = ps_big.tile([128, 512], F32, tag="big")
            for eb in range(NEB):
                nc.tensor.matmul(
                    pp[:, :],
                    KT[eb][:, lbg * 128:(lbg + 1) * 128],
                    wk_sb[eb][:, :],
                    start=(eb == 0), stop=False)
            nc.tensor.matmul(pp[:, :], ones_row[:, :], bk2_sb[:, :],
                             start=False, stop=True)
            nc.scalar.activation(
                out=sigK[:, lbg * E:(lbg + 1) * E], in_=pp[:, :],
                func=AF.Sigmoid, scale=2.0)
        if ci == DEFER - 1:
            build_qgbd()
        if ci >= DEFER:
            deferred_stage(ci - DEFER)
    for ci in range(NCH - DEFER, NCH):
        deferred_stage(ci)

    # ---------------- phase 7: KV exchange + KV2 build ----------------
    kv_sb = misc.tile([128, E], F32, tag="kvsb")
    nc.vector.tensor_copy(kv_sb[:, :], kv_psum[:, :])
    ks_sb = misc.tile([1, E], F32, tag="kssb")
    nc.vector.tensor_copy(ks_sb[:, :], ksum_psum[:, :])
    nc.sync.dma_start(out=kv_in.ap()[0:128, :], in_=kv_sb[:, :])
    nc.sync.dma_start(out=kv_in.ap()[128:129, :], in_=ks_sb[:, :])
    nc.gpsimd.collective_compute(
        "AllReduce", ALU.add, replica_groups=GROUPS,
        ins=[kv_in.ap().opt()], outs=[kv_out.ap().opt()])
    kvg_sb = misc.tile([128, E], F32, tag="kvg")
    nc.sync.dma_start(out=kvg_sb[:, :], in_=kv_out.ap()[0:128, :])
    ksc = misc.tile([128, NEB], F32, tag="kscf")
    for p in range(NEB):
        nc.sync.dma_start(
            out=ksc[0:64, p:p + 1],
            in_=kv_out.ap()[128:129, p * 128:p * 128 + 64].rearrange(
                "a b -> b a"))
        nc.sync.dma_start(
            out=ksc[64:128, p:p + 1],
            in_=kv_out.ap()[128:129, p * 128 + 64:(p + 1) * 128].rearrange(
                "a b -> b a"))
    # the per-head normalizers cancel between num and den, so KV2 is just
    # the bf16 blockdiag re-layout of the raw AllReduce output.
    kv2 = []
    for p in range(NEB):
        t = singles.tile([128, 130], BF16, tag=f"kv2_{p}", name=f"kv2_{p}")
        nc.vector.memset(t, 0.0)
        nc.vector.tensor_copy(t[0:64, 0:64],
                              kvg_sb[0:64, p * 128:p * 128 + 64])
        nc.vector.tensor_copy(t[64:128, 64:128],
                              kvg_sb[64:128, p * 128 + 64:(p + 1) * 128])
        nc.vector.tensor_copy(t[0:64, 128:129], ksc[0:64, p:p + 1])
        nc.vector.tensor_copy(t[64:128, 129:130], ksc[64:128, p:p + 1])
        kv2.append(t)

    # ---------------- phase 8: num / den / out ----------------
    den = None
    rec = None
    for lbg in range(NLB):
        num = ps_big.tile([128, E], F32, tag="big")
        if lbg % 4 == 0:
            den = ps_small.tile([128, 4 * H], F32, tag="small")
            rec = recp.tile([128, 4 * H], F32, tag="rec")
        for p in range(NEB):
            lhs = sigQT[p][:, lbg * 128:(lbg + 1) * 128]
            nc.tensor.matmul(num[:, p * 128:(p + 1) * 128], lhs,
                             kv2[p][:, 0:128], start=True, stop=True,
                             skip_group_check=True)
            nc.tensor.matmul(den[:, (lbg % 4) * H + 2 * p:
                                 (lbg % 4) * H + 2 * p + 2], lhs,
                             kv2[p][:, 128:130], start=True, stop=True,
                             skip_group_check=True)
        dsl = slice((lbg % 4) * H, (lbg % 4 + 1) * H)
        nc.vector.reciprocal(rec[:, dsl], den[:, dsl])
        ot = outp.tile([128, E], F32, tag="ot")
        if lbg % 2 == 0:
            # DVE path: one broadcast tensor_tensor from PSUM
            nc.vector.tensor_mul(
                ot.rearrange("p (h d) -> p h d", h=H),
                num.rearrange("p (h d) -> p h d", h=H),
                rec[:, (lbg % 4) * H:(lbg % 4 + 1) * H].unsqueeze(
                    2).broadcast_to([128, H, D]))
        else:
            # ACT path (idle in this phase): per-head Copy with a
            # per-partition scale AP does out = rec[l,h] * num[l, h*D:...]
            for h in range(H):
                nc.scalar.activation(
                    out=ot[:, h * D:(h + 1) * D],
                    in_=num[:, h * D:(h + 1) * D],
                    func=AF.Copy,
                    scale=rec[:, (lbg % 4) * H + h:(lbg % 4) * H + h + 1])
        nc.sync.dma_start(out=Out.ap()[lbg * 128:(lbg + 1) * 128, :],
                          in_=ot[:, :])

    if dbg is not None:
        nc.gpsimd.dma_start(out=dbg["dbg_qg"].ap()[:, :], in_=qg_out.ap()[:, :])
        nc.sync.dma_start(out=dbg["dbg_sigqt0"].ap()[:, :], in_=sigQT[0][:, :])
        nc.sync.dma_start(out=dbg["dbg_sigk0"].ap()[:, :], in_=sigK[:, 0:E])
        nc.gpsimd.dma_start(out=dbg["dbg_expt"].ap()[:, :], in_=expT_b[:, :])
        nc.gpsimd.dma_start(out=dbg["dbg_kvout"].ap()[:, :],
                            in_=kv_out.ap()[:, :])


_NC_CACHE = None


def _get_nc():
    global _NC_CACHE
    if _NC_CACHE is None:
        _NC_CACHE = _build()
    return _NC_CACHE


def kernel(Q, K, V, Wq, bq, Wk, bk):
    Q = np.asarray(Q, dtype=np.float32)
    K = np.asarray(K, dtype=np.float32)
    V = np.asarray(V, dtype=np.float32)
    WqT_np = np.ascontiguousarray(np.asarray(Wq, np.float32).T).astype(
        ml_dtypes.bfloat16)
    WkT_np = np.ascontiguousarray(np.asarray(Wk, np.float32).T).astype(
        ml_dtypes.bfloat16)
    bq2_np = 2.0 * np.asarray(bq, np.float32)
    bk2_np = (2.0 * np.asarray(bk, np.float32)).reshape(1, E).astype(
        ml_dtypes.bfloat16)

    nc = _get_nc()
    in_maps = []
    for c in range(N_CORES):
        b, half = c // 2, c % 2
        lo = half * LH
        in_maps.append({
            "Qs": np.ascontiguousarray(Q[b, lo:lo + LH, :]),
            "Ks": np.ascontiguousarray(K[b, lo:lo + LH, :]),
            "Vs": np.ascontiguousarray(V[b, lo:lo + LH, :]),
            "WqT": WqT_np, "WkT": WkT_np, "bq2": bq2_np, "bk2": bk2_np,
        })
    res = run_bass_kernel_spmd(nc, in_maps, core_ids=list(range(N_CORES)))
    out = np.empty((B, L, E), np.float32)
    for c in range(N_CORES):
        b, half = c // 2, c % 2
        out[b, half * LH:(half + 1) * LH, :] = res.results[c]["Out"]
    return out


# revision 22
# speedup vs baseline: 1.1859x; 1.1859x over previous
"""Trainium2 Bass kernel for MultiHeadLinearAttention (RALA / tanh-kernel linear attention).

Reference computation (B=4, L=8192, E=512, H=8, D=64):
    phi_Q = tanh(Q @ Wq.T + bq) + 1 ;  phi_K = tanh(K @ Wk.T + bk) + 1
    q_global = mean_L(Qh) / sqrt(D)
    alpha = softmax_L(q_global . Kh) * L
    KV[h]  = sum_l (phi_Kh*alpha) (x) Vh ;  K_sum[h] = sum_l phi_Kh*alpha
    out = (phi_Qh @ KV) / (phi_Qh . K_sum + eps)

Sharding: 8 cores = (batch b in 0..3) x (sequence half in 0..1).  Every
input element is read exactly once.  The L-reductions (q_global, KV,
K_sum) are completed with tiny pairwise AllReduces between the two cores
sharing a batch.

Identities used on-chip:
    tanh(x+b)+1 = 2*sigmoid(2x+2b)      (single ACT pass from PSUM)
    exp(x) = s/(1-s) with s=sigmoid(x)  (stays on the sigmoid table set)
    The per-head scalars (the 2x factors and the softmax normalizer
    L/S_h) multiply BOTH num and den, so they cancel in out=num/den and
    are never computed.  eps=1e-6 is dropped: den ~ 1e5 before any
    normalization, eps is far below its fp32 ulp.
    softmax max-subtraction is skipped: |scores| ~ 0.01.

Compute dtype: bf16 matmul inputs with fp32 PSUM accumulation.
"""

import numpy as np
import ml_dtypes

import concourse.bass as bass
from concourse import bacc
import concourse.mybir as mybir
import concourse.tile as tile
from concourse.bass_utils import run_bass_kernel_spmd
from concourse.masks import make_identity

B, L, E, H, D = 4, 8192, 512, 8, 64
N_CORES = 8
LH = L // 2            # 4096 rows per core
NLB = LH // 128        # 32 l-blocks of 128 rows
NCH = LH // 512        # 8 chunks of 512 rows
NEB = E // 128         # 4 e-blocks (= 4 head-pairs = 4 j-blocks)
GROUPS = [[0, 1], [2, 3], [4, 5], [6, 7]]
F32 = mybir.dt.float32
BF16 = mybir.dt.bfloat16
AF = mybir.ActivationFunctionType
ALU = mybir.AluOpType
QG_SCALE = 1.0 / (L * np.sqrt(D))   # scores = (sum_l Q) . K * QG_SCALE

DEBUG = False


def _build():
    nc = bacc.Bacc("TRN2", target_bir_lowering=False, debug=False,
                   num_devices=N_CORES)
    Qs = nc.dram_tensor("Qs", [LH, E], F32, kind="ExternalInput")
    Ks = nc.dram_tensor("Ks", [LH, E], F32, kind="ExternalInput")
    Vs = nc.dram_tensor("Vs", [LH, E], F32, kind="ExternalInput")
    WqT = nc.dram_tensor("WqT", [E, E], BF16, kind="ExternalInput")
    WkT = nc.dram_tensor("WkT", [E, E], BF16, kind="ExternalInput")
    bq2 = nc.dram_tensor("bq2", [E], F32, kind="ExternalInput")
    bk2 = nc.dram_tensor("bk2", [1, E], BF16, kind="ExternalInput")
    Out = nc.dram_tensor("Out", [LH, E], F32, kind="ExternalOutput")

    # DRAM bounce buffers for the two pairwise AllReduces
    qg_in = nc.dram_tensor("qg_in", [1, E], F32)
    qg_out = nc.dram_tensor("qg_out", [1, E], F32)
    kv_in = nc.dram_tensor("kv_in", [129, E], F32)
    kv_out = nc.dram_tensor("kv_out", [129, E], F32)
    dbg = None
    if DEBUG:
        dbg = {
            "dbg_qg": nc.dram_tensor("dbg_qg", [1, E], F32, kind="ExternalOutput"),
            "dbg_sigqt0": nc.dram_tensor("dbg_sigqt0", [128, LH], BF16, kind="ExternalOutput"),
            "dbg_sigk0": nc.dram_tensor("dbg_sigk0", [128, E], BF16, kind="ExternalOutput"),
            "dbg_expt": nc.dram_tensor("dbg_expt", [128, NLB * H], F32, kind="ExternalOutput"),
            "dbg_kvout": nc.dram_tensor("dbg_kvout", [129, E], F32, kind="ExternalOutput"),
        }

    import contextlib
    with tile.TileContext(nc) as tc:
        with contextlib.ExitStack() as ctx:
            _kernel_body(nc, tc, ctx, Qs, Ks, Vs, WqT, WkT, bq2, bk2, Out,
                         qg_in, qg_out, kv_in, kv_out, dbg)
    nc.finalize()
    return nc


def _kernel_body(nc, tc, ctx, Qs, Ks, Vs, WqT, WkT, bq2, bk2, Out,
                 qg_in, qg_out, kv_in, kv_out, dbg=None):
    enter = ctx.enter_context

    # ---------------- pools ----------------
    singles = enter(tc.tile_pool(name="singles", bufs=1))
    stream = enter(tc.tile_pool(name="stream", bufs=6))       # Q/K/V nat tiles
    qt_pool = enter(tc.tile_pool(name="qt", bufs=2))          # QT chunk tiles
    outp = enter(tc.tile_pool(name="outp", bufs=6))           # out tiles
    recp = enter(tc.tile_pool(name="recp", bufs=8))
    misc = enter(tc.tile_pool(name="misc", bufs=1))
    # PSUM budget (8 banks):  big 2 + tp 2 + small 2 + kv 1 + accum1 1
    ps_big = enter(tc.tile_pool(name="ps_big", bufs=2, space="PSUM"))
    ps_tp = enter(tc.tile_pool(name="ps_tp", bufs=2, space="PSUM"))
    ps_small = enter(tc.tile_pool(name="ps_small", bufs=2, space="PSUM"))
    ps_kv = enter(tc.tile_pool(name="ps_kv", bufs=1, space="PSUM"))
    ps_acc = enter(tc.tile_pool(name="ps_acc", bufs=1, space="PSUM"))

    # ---------------- constants ----------------
    ident = singles.tile([128, 128], BF16, tag="ident")
    make_identity(nc, ident)
    ones_col = singles.tile([128, 1], BF16, tag="ones_col")
    nc.vector.memset(ones_col, 1.0)
    ones_row = singles.tile([1, 128], BF16, tag="ones_row")
    nc.vector.memset(ones_row, 1.0)

    # weights / biases
    wq_sb = []
    wk_sb = []
    for eb in range(NEB):
        wq = singles.tile([128, E], BF16, tag=f"wq{eb}", name=f"wq{eb}")
        nc.sync.dma_start(out=wq[:, :], in_=WqT.ap()[eb * 128:(eb + 1) * 128, :])
        wq_sb.append(wq)
        wk = singles.tile([128, E], BF16, tag=f"wk{eb}", name=f"wk{eb}")
        nc.sync.dma_start(out=wk[:, :], in_=WkT.ap()[eb * 128:(eb + 1) * 128, :])
        wk_sb.append(wk)
    bq2_sb = singles.tile([128, NEB], F32, tag="bq2")
    nc.sync.dma_start(
        out=bq2_sb[:, :],
        in_=bass.AP(tensor=bq2, offset=0, ap=[[1, 128], [128, NEB]]))
    bk2_sb = singles.tile([1, E], BF16, tag="bk2")
    nc.sync.dma_start(out=bk2_sb[:, :], in_=bk2.ap()[:, :])

    # ---------------- persistent SBUF tensors ----------------
    # sigQT[jb]: [128 j, LH l] bf16 ; KT[eb]: [128 e, LH l] bf16
    sigQT = [singles.tile([128, LH], BF16, tag=f"sigqt{j}", name=f"sigqt{j}")
             for j in range(NEB)]
    KT = [singles.tile([128, LH], BF16, tag=f"kt{e}", name=f"kt{e}")
          for e in range(NEB)]
    sigK = singles.tile([128, NLB * E], BF16, tag="sigk")   # [l, 512 j] per blk
    expS = singles.tile([128, NLB * H], F32, tag="exps")    # sigmoid(scores)
    expT_b = singles.tile([128, NLB * H], BF16, tag="exptb")
    # qg partials: accum_out of the QT-copy per (eb, chunk)
    qg_acc = [singles.tile([128, NCH], F32, tag=f"qgacc{e}", name=f"qgacc{e}")
              for e in range(NEB)]

    # ---------------- phase 1: Q pass ----------------
    # K loads are emitted interleaved with the Q chunks (kn has 32 bufs,
    # so all of K prefetches during phase 1 and the K compute can start
    # the moment the last phi_Q matmul drains).
    kns_all = []
    for ci in range(NCH):
        qns = []
        for lb in range(4):
            lbg = ci * 4 + lb
            qn = stream.tile([128, E], BF16, tag="qn")
            nc.gpsimd.dma_start(out=qn[:, :],
                                in_=Qs.ap()[lbg * 128:(lbg + 1) * 128, :])
            qns.append(qn)
        kns = []
        for lb in range(4):
            lbg = ci * 4 + lb
            kn = stream.tile([128, E], BF16, tag="kn", bufs=32)
            nc.gpsimd.dma_start(out=kn[:, :],
                                in_=Ks.ap()[lbg * 128:(lbg + 1) * 128, :])
            kns.append(kn)
        kns_all.append(kns)
        qt_tiles = []
        for eb in range(NEB):
            tp = ps_tp.tile([128, 512], BF16, tag="tp")
            for lb in range(4):
                nc.tensor.transpose(tp[:, lb * 128:(lb + 1) * 128],
                                    qns[lb][:, eb * 128:(eb + 1) * 128],
                                    ident[:, :])
            qt = qt_pool.tile([128, 512], BF16, tag=f"qt{eb}", name=f"qtt{eb}")
            # copy PSUM->SBUF on DVE; accum_out gives sum over l for free,
            # accumulating the q_global partial per (e, chunk).
            nc.vector.tensor_scalar(
                out=qt[:, :], in0=tp[:, :], scalar1=0.0, scalar2=0.0,
                op0=ALU.add, op1=ALU.add,
                accum_out=qg_acc[eb][:, ci:ci + 1])
            qt_tiles.append(qt)
        for jb in range(NEB):
            pp = ps_big.tile([128, 512], F32, tag="big")
            for eb in range(NEB):
                nc.tensor.matmul(pp[:, :],
                                 wq_sb[eb][:, jb * 128:(jb + 1) * 128],
                                 qt_tiles[eb][:, :],
                                 start=(eb == 0), stop=(eb == NEB - 1))
            nc.scalar.activation(
                out=sigQT[jb][:, ci * 512:(ci + 1) * 512], in_=pp[:, :],
                func=AF.Sigmoid, bias=bq2_sb[:, jb:jb + 1], scale=2.0)

    # ---------------- phase 2: qg exchange ----------------
    for eb in range(NEB):
        qgc = misc.tile([128, 1], F32, tag=f"qgc{eb}", name=f"qgc{eb}")
        nc.vector.reduce_sum(qgc[:, :], qg_acc[eb][:, :],
                             axis=mybir.AxisListType.X)
        nc.sync.dma_start(
            out=qg_in.ap()[0:1, eb * 128:(eb + 1) * 128].rearrange(
                "a b -> b a"),
            in_=qgc[:, :])
    nc.gpsimd.collective_compute(
        "AllReduce", ALU.add, replica_groups=GROUPS,
        ins=[qg_in.ap().opt()], outs=[qg_out.ap().opt()])
    qgbd = [None] * NEB

    def build_qgbd():
        # blockdiag q_global stationaries [128, 2] bf16 per head-pair.
        # Loads go over HWDGE (sync) so they don't head-of-line-block the
        # gpsimd queue carrying the V cast-loads; DVE does the f32->bf16.
        qgbd_f = misc.tile([128, 2 * NEB], F32, tag="qgbdf", name="qgbdf")
        for p in range(NEB):
            nc.sync.dma_start(
                out=qgbd_f[0:64, 2 * p:2 * p + 1],
                in_=qg_out.ap()[0:1, p * 128:p * 128 + 64].rearrange(
                    "a b -> b a"))
            nc.sync.dma_start(
                out=qgbd_f[64:128, 2 * p + 1:2 * p + 2],
                in_=qg_out.ap()[0:1, p * 128 + 64:(p + 1) * 128].rearrange(
                    "a b -> b a"))
            t = singles.tile([128, 2], BF16, tag=f"qgbd{p}", name=f"qgbd{p}")
            nc.vector.memset(t, 0.0)
            nc.vector.tensor_copy(t[0:64, 0:1], qgbd_f[0:64, 2 * p:2 * p + 1])
            nc.vector.tensor_copy(t[64:128, 1:2],
                                  qgbd_f[64:128, 2 * p + 1:2 * p + 2])
            qgbd[p] = t

    # ------- phases 3-6 fused: K/V streaming pass -------
    # Main stage per chunk: transposes -> KT, phi_K.  The qg-dependent
    # stage (scoresT, exp, phiKs scale, V/KV accumulation) is emitted
    # DEFER chunks later so the engine queues never stall on the qg
    # AllReduce.
    DEFER = 3
    kv_psum = ps_kv.tile([128, E], F32, tag="kv")
    ksum_psum = ps_acc.tile([1, E], F32, tag="acc1")

    def deferred_stage(ci):
        for lb in range(4):
            lbg = ci * 4 + lb
            sc = ps_small.tile([128, H], F32, tag="small", name="sc")
            for p in range(NEB):
                nc.tensor.matmul(sc[:, 2 * p:2 * p + 2],
                                 KT[p][:, lbg * 128:(lbg + 1) * 128],
                                 qgbd[p][:, :], start=True, stop=True,
                                 skip_group_check=True)
            nc.scalar.activation(out=expS[:, lbg * H:(lbg + 1) * H],
                                 in_=sc[:, :], func=AF.Sigmoid,
                                 scale=float(QG_SCALE))
        # exp = s/(1-s) for the whole chunk [128, 32]
        cs = slice(ci * 4 * H, (ci + 1) * 4 * H)
        om = misc.tile([128, 4 * H], F32, tag="om", name="om")
        nc.vector.tensor_scalar(out=om[:, :], in0=expS[:, cs], scalar1=-1.0,
                                scalar2=1.0, op0=ALU.mult, op1=ALU.add)
        nc.vector.reciprocal(om[:, :], om[:, :])
        nc.vector.tensor_mul(expT_b[:, cs], expS[:, cs], om[:, :])
        # phiKs = sigK * expT for the whole chunk (one strided TT)
        sig_sl = sigK[:, ci * 4 * E:(ci + 1) * 4 * E].rearrange(
            "p (t h d) -> p t h d", t=4, h=H)
        eb_ap = expT_b[:, cs].rearrange("p (t h) -> p t h", t=4).unsqueeze(
            3).broadcast_to([128, 4, H, D])
        nc.vector.tensor_mul(sig_sl, sig_sl, eb_ap)
        # V / KV accumulation for this chunk
        for lb in range(4):
            lbg = ci * 4 + lb
            vn = stream.tile([128, E], BF16, tag="vn", name="vn")
            nc.gpsimd.dma_start(out=vn[:, :],
                                in_=Vs.ap()[lbg * 128:(lbg + 1) * 128, :])
            for p in range(NEB):
                # start=True clears has_written for the WHOLE bank: only the
                # very first matmul into this bank may use it.
                nc.tensor.matmul(
                    kv_psum[:, p * 128:(p + 1) * 128],
                    sigK[:, lbg * E + p * 128:lbg * E + (p + 1) * 128],
                    vn[:, p * 128:(p + 1) * 128],
                    start=(lbg == 0 and p == 0), stop=(lbg == NLB - 1),
                    skip_group_check=True)
            nc.tensor.matmul(ksum_psum[:, :], ones_col[:, :],
                             sigK[:, lbg * E:(lbg + 1) * E],
                             start=(lbg == 0), stop=(lbg == NLB - 1),
                             skip_group_check=True)

    for ci in range(NCH):
        kns = kns_all[ci]
        for eb in range(NEB):
            tp = ps_tp.tile([128, 512], BF16, tag="tp")
            for lb in range(4):
                nc.tensor.transpose(tp[:, lb * 128:(lb + 1) * 128],
                                    kns[lb][:, eb * 128:(eb + 1) * 128],
                                    ident[:, :])
            nc.vector.tensor_copy(KT[eb][:, ci * 512:(ci + 1) * 512], tp[:, :])
        for lb in range(4):
            lbg = ci * 4 + lb
            pp = ps_big.tile([128, 512], F32, tag="big")
            for eb in range(NEB):
                nc.tensor.matmul(
                    pp[:, :],
                    KT[eb][:, lbg * 128:(lbg + 1) * 128],
                    wk_sb[eb][:, :],
                    start=(eb == 0), stop=False)
            nc.tensor.matmul(pp[:, :], ones_row[:, :], bk2_sb[:, :],
                             start=False, stop=True)
            nc.scalar.activation(
                out=sigK[:, lbg * E:(lbg + 1) * E], in_=pp[:, :],
                func=AF.Sigmoid, scale=2.0)
        if ci == DEFER - 1:
            build_qgbd()
        if ci >= DEFER:
            deferred_stage(ci - DEFER)
    for ci in range(NCH - DEFER, NCH):
        deferred_stage(ci)

    # ---------------- phase 7: KV exchange + KV2 build ----------------
    kv_sb = misc.tile([128, E], F32, tag="kvsb")
    nc.vector.tensor_copy(kv_sb[:, :], kv_psum[:, :])
    ks_sb = misc.tile([1, E], F32, tag="kssb")
    nc.vector.tensor_copy(ks_sb[:, :], ksum_psum[:, :])
    nc.sync.dma_start(out=kv_in.ap()[0:128, :], in_=kv_sb[:, :])
    nc.sync.dma_start(out=kv_in.ap()[128:129, :], in_=ks_sb[:, :])
    nc.gpsimd.collective_compute(
        "AllReduce", ALU.add, replica_groups=GROUPS,
        ins=[kv_in.ap().opt()], outs=[kv_out.ap().opt()])
    kvg_sb = misc.tile([128, E], F32, tag="kvg")
    nc.sync.dma_start(out=kvg_sb[:, :], in_=kv_out.ap()[0:128, :])
    ksc = misc.tile([128, NEB], F32, tag="kscf")
    for p in range(NEB):
        nc.sync.dma_start(
            out=ksc[0:64, p:p + 1],
            in_=kv_out.ap()[128:129, p * 128:p * 128 + 64].rearrange(
                "a b -> b a"))
        nc.sync.dma_start(
            out=ksc[64:128, p:p + 1],
            in_=kv_out.ap()[128:129, p * 128 + 64:(p + 1) * 128].rearrange(
                "a b -> b a"))
    # the per-head normalizers cancel between num and den, so KV2 is just
    # the bf16 blockdiag re-layout of the raw AllReduce output.
    kv2 = []
    for p in range(NEB):
        t = singles.tile([128, 130], BF16, tag=f"kv2_{p}", name=f"kv2_{p}")
        nc.vector.memset(t, 0.0)
        nc.vector.tensor_copy(t[0:64, 0:64],
                              kvg_sb[0:64, p * 128:p * 128 + 64])
        nc.vector.tensor_copy(t[64:128, 64:128],
                              kvg_sb[64:128, p * 128 + 64:(p + 1) * 128])
        nc.vector.tensor_copy(t[0:64, 128:129], ksc[0:64, p:p + 1])
        nc.vector.tensor_copy(t[64:128, 129:130], ksc[64:128, p:p + 1])
        kv2.append(t)

    # ---------------- phase 8: num / den / out ----------------
    den = None
    rec = None
    for lbg in range(NLB):
        num = ps_big.tile([128, E], F32, tag="big")
        if lbg % 4 == 0:
            den = ps_small.tile([128, 4 * H], F32, tag="small")
            rec = recp.tile([128, 4 * H], F32, tag="rec")
        for p in range(NEB):
            lhs = sigQT[p][:, lbg * 128:(lbg + 1) * 128]
            nc.tensor.matmul(num[:, p * 128:(p + 1) * 128], lhs,
                             kv2[p][:, 0:128], start=True, stop=True,
                             skip_group_check=True)
            nc.tensor.matmul(den[:, (lbg % 4) * H + 2 * p:
                                 (lbg % 4) * H + 2 * p + 2], lhs,
                             kv2[p][:, 128:130], start=True, stop=True,
                             skip_group_check=True)
        dsl = slice((lbg % 4) * H, (lbg % 4 + 1) * H)
        nc.vector.reciprocal(rec[:, dsl], den[:, dsl])
        ot = outp.tile([128, E], F32, tag="ot")
        nc.vector.tensor_mul(
            ot.rearrange("p (h d) -> p h d", h=H),
            num.rearrange("p (h d) -> p h d", h=H),
            rec[:, (lbg % 4) * H:(lbg % 4 + 1) * H].unsqueeze(
                2).broadcast_to([128, H, D]))
        nc.sync.dma_start(out=Out.ap()[lbg * 128:(lbg + 1) * 128, :],
                          in_=ot[:, :])

    if dbg is not None:
        nc.gpsimd.dma_start(out=dbg["dbg_qg"].ap()[:, :], in_=qg_out.ap()[:, :])
        nc.sync.dma_start(out=dbg["dbg_sigqt0"].ap()[:, :], in_=sigQT[0][:, :])
        nc.sync.dma_start(out=dbg["dbg_sigk0"].ap()[:, :], in_=sigK[:, 0:E])
        nc.gpsimd.dma_start(out=dbg["dbg_expt"].ap()[:, :], in_=expT_b[:, :])
        nc.gpsimd.dma_start(out=dbg["dbg_kvout"].ap()[:, :],
                            in_=kv_out.ap()[:, :])


_NC_CACHE = None


def _get_nc():
    global _NC_CACHE
    if _NC_CACHE is None:
        _NC_CACHE = _build()
    return _NC_CACHE


def kernel(Q, K, V, Wq, bq, Wk, bk):
    Q = np.asarray(Q, dtype=np.float32)
    K = np.asarray(K, dtype=np.float32)
    V = np.asarray(V, dtype=np.float32)
    WqT_np = np.ascontiguousarray(np.asarray(Wq, np.float32).T).astype(
        ml_dtypes.bfloat16)
    WkT_np = np.ascontiguousarray(np.asarray(Wk, np.float32).T).astype(
        ml_dtypes.bfloat16)
    bq2_np = 2.0 * np.asarray(bq, np.float32)
    bk2_np = (2.0 * np.asarray(bk, np.float32)).reshape(1, E).astype(
        ml_dtypes.bfloat16)

    nc = _get_nc()
    in_maps = []
    for c in range(N_CORES):
        b, half = c // 2, c % 2
        lo = half * LH
        in_maps.append({
            "Qs": np.ascontiguousarray(Q[b, lo:lo + LH, :]),
            "Ks": np.ascontiguousarray(K[b, lo:lo + LH, :]),
            "Vs": np.ascontiguousarray(V[b, lo:lo + LH, :]),
            "WqT": WqT_np, "WkT": WkT_np, "bq2": bq2_np, "bk2": bk2_np,
        })
    res = run_bass_kernel_spmd(nc, in_maps, core_ids=list(range(N_CORES)))
    out = np.empty((B, L, E), np.float32)
    for c in range(N_CORES):
        b, half = c // 2, c % 2
        out[b, half * LH:(half + 1) * LH, :] = res.results[c]["Out"]
    return out
